# revision 27
# baseline (speedup 1.0000x reference)
"""Trainium2 Bass kernel for the dual-stream transformer block.

Shapes (hardcoded): B=4, S=1024, D=768, F=3072, H=12, DH=64, 8 cores.
Sharding: data-parallel; core c handles (stream = c//4, batch = c%4) and
computes that stream's full [S, D] output. Cross-attention K/V for the other
stream is recomputed locally (no collectives). The w11/w12-style stream-mix
scalars are folded into the V projection weights host-side (linearity), and
softmax normalizers come from a ones-column augmented V.
"""
import sys, os

sys.path.insert(0, "/opt/trn_rl_repo")

ABL = set(os.environ.get("ABL", "").split(",")) - {""}
NREP = int(os.environ.get("NREP", "1"))

import numpy as np
import ml_dtypes

import concourse.bass as bass
import concourse.mybir as mybir
import concourse.tile as tile
from concourse import bacc
from concourse.bass import ts, ds
from concourse.masks import make_identity

F32 = mybir.dt.float32
BF16 = mybir.dt.bfloat16
AF = mybir.ActivationFunctionType
ALU = mybir.AluOpType

B = 4
S = 1024
D = 768
FF = 3072
H = 12
DH = 64
P = 128
NDC = D // P          # 6 chunks of D
NF = FF // P          # 24 chunks of F
NT = S // P           # 8 token tiles
NHP = H // 2          # 6 head pairs
QC = 512              # q-chunk (attention moving free dim)
NQC = S // QC         # 2
EPS = 1e-6
N_CORES = 8

VAUG = DH + 1         # 65: v columns + ones column
VW = 2 * VAUG         # 130 per head pair


def _bcast_ap(ap, n):
    """Partition-broadcast read AP: replicate across n partitions."""
    dims = list(ap.ap)
    if dims and dims[0][1] == 1:
        dims = dims[1:]
    return bass.AP(tensor=ap.tensor, offset=ap.offset, ap=[[0, n]] + dims)


def build_program():
    nc = bacc.Bacc("TRN2", target_bir_lowering=False, debug=False,
                   num_devices=N_CORES)

    dt_in = {}

    def din(name, shape, dt=F32):
        h = nc.dram_tensor(name, shape, dt, kind="ExternalInput")
        dt_in[name] = h
        return h

    din("xin", [S, D])
    din("oin", [S, D])
    din("cols", [P, 9 * NDC + NF])
    din("wq", [D, D], BF16)
    din("wk", [D, D], BF16)
    din("wko", [D, D], BF16)
    din("wvaug", [D, H * VAUG], BF16)
    din("wvoaug", [D, H * VAUG], BF16)

    din("wo", [D, D], BF16)

    din("w1", [D, FF], BF16)

    din("w2", [FF, D], BF16)
    din("brows", [2 * H * VAUG + 2 * D])
    out = nc.dram_tensor("out", [S, D], F32, kind="ExternalOutput")

    views = {
        "wq_v": dt_in["wq"].ap().rearrange("(c kp) n -> kp c n", kp=P),
        "wk_v": dt_in["wk"].ap().rearrange("(c kp) n -> kp c n", kp=P),
        "wko_v": dt_in["wko"].ap().rearrange("(c kp) n -> kp c n", kp=P),
        "wva_v": dt_in["wvaug"].ap().rearrange("(c kp) n -> kp c n", kp=P),
        "wvo_v": dt_in["wvoaug"].ap().rearrange("(c kp) n -> kp c n", kp=P),
        "wo_v": dt_in["wo"].ap().rearrange("(c kp) n -> kp c n", kp=P),
        "w1_v": dt_in["w1"].ap().rearrange("(c kp) n -> kp c n", kp=P),
        "w2_v": dt_in["w2"].ap().rearrange("(c kp) n -> kp c n", kp=P),
    }

    with tile.TileContext(nc) as tc:
        for _rep in range(NREP):
            _emit(nc, tc, dt_in, views, out)
    nc.compile()
    return nc


def _emit(nc, tc, dt_in, views, out):
    from contextlib import ExitStack

    with ExitStack() as ctx:
        glob = ctx.enter_context(tc.tile_pool(name="glob", bufs=1))
        acts1 = None  # opened below, closed after phase D
        lnp = ctx.enter_context(tc.tile_pool(name="lnp", bufs=2))
        lns = ctx.enter_context(tc.tile_pool(name="lns", bufs=4))

        ident = glob.tile([P, P], BF16)
        make_identity(nc, ident)
        eps_t = glob.tile([P, 1], F32)
        nc.vector.memset(eps_t, EPS)
        ones_t = glob.tile([1, DH], BF16)
        nc.vector.memset(ones_t, 1.0)

        cols_t = glob.tile([P, 9 * NDC + NF], F32)
        nc.sync.dma_start(cols_t, dt_in["cols"].ap())
        g1_t, b1_t = cols_t[:, 0:6], cols_t[:, 6:12]
        g1o_t, b1o_t = cols_t[:, 12:18], cols_t[:, 18:24]
        g2_t, b2_t = cols_t[:, 24:30], cols_t[:, 30:36]
        bq_t, bk_t, bko_t = cols_t[:, 36:42], cols_t[:, 42:48], cols_t[:, 48:54]
        b1f_t = cols_t[:, 54:78]

        brows_t = glob.tile([P, 2 * H * VAUG + 2 * D], F32)
        nc.sync.dma_start(brows_t, _bcast_ap(dt_in["brows"].ap(), P))
        HV = H * VAUG
        bva_t = brows_t[:, 0:HV]
        bvo_t = brows_t[:, HV:2 * HV]
        boeff_t = brows_t[:, 2 * HV:2 * HV + D]
        b2f_t = brows_t[:, 2 * HV + D:2 * HV + 2 * D]

        w2p = ctx.enter_context(tc.tile_pool(name="w2p", bufs=1))
        w2s = w2p.tile([P, NF, D], BF16)
        dwp = ctx.enter_context(tc.tile_pool(name="dwp", bufs=1))
        wo_t = dwp.tile([P, NDC, D], BF16)

        # ---------- LayerNorm helper (token-major stats, transposed output) ----
        def layernorm_T(src_getter, g_t, b_t, xnT, tps, cb_act=True):
            # xnT: callable ti -> half tile [P, NDC, QC]
            # cb_act: put the transpose copyback (with g/b) on ACT vs DVE
            for ti in range(NT):
                xt = src_getter(ti)
                stats = lns.tile([P, 3, 6], F32, tag="stats")
                for sg in range(3):
                    nc.vector.bn_stats(out=stats[:, sg, :],
                                       in_=xt[:, ds(sg * 256, 256)])
                mv = lns.tile([P, 2], F32, tag="mv")
                nc.vector.bn_aggr(out=mv, in_=stats)
                rstd = lns.tile([P, 1], F32, tag="rstd")
                nc.scalar.activation(out=rstd, in_=mv[:, 1:2], func=AF.Sqrt,
                                     bias=eps_t, scale=1.0)
                nc.vector.reciprocal(out=rstd, in_=rstd)
                nmr = lns.tile([P, 1], F32, tag="nmr")
                nc.vector.tensor_tensor(nmr, mv[:, 0:1], rstd, ALU.mult)
                nc.vector.tensor_scalar_mul(nmr, nmr, -1.0)
                xn = lnp.tile([P, D], BF16, tag="xn")
                nc.scalar.activation(out=xn, in_=xt, func=AF.Identity,
                                     bias=nmr, scale=rstd)
                for c in range(NDC):
                    ps = tps.tile([P, P], BF16, tag="tp")
                    nc.tensor.transpose(ps, xn[:, ts(c, P)], ident)
                    if c % 2 == (0 if cb_act else 1):
                        nc.scalar.activation(
                            out=xnT(ti)[:, c, ts(ti % 4, P)], in_=ps,
                            func=AF.Identity, bias=b_t[:, c:c + 1],
                            scale=g_t[:, c:c + 1])
                    else:
                        nc.vector.tensor_scalar(
                            out=xnT(ti)[:, c, ts(ti % 4, P)], in0=ps,
                            scalar1=g_t[:, c:c + 1], scalar2=b_t[:, c:c + 1],
                            op0=ALU.mult, op1=ALU.add)

        # ---------- Phase A: LN1 for both streams ----------
        # xnT stored as two S-half tiles so consumers can start after half LN
        acts1 = ctx.enter_context(tc.tile_pool(name="acts1", bufs=1))
        xnT_own_h = [acts1.tile([P, NDC, QC], BF16, tag=f"xnT_own{h}",
                                name=f"xnT_own{h}") for h in range(2)]
        xnT_oth_h = [acts1.tile([P, NDC, QC], BF16, tag=f"xnT_oth{h}",
                                name=f"xnT_oth{h}") for h in range(2)]
        ctxT = acts1.tile([P, NDC, S], BF16, tag="ctxT")

        def mk_src(dram):
            def get(ti):
                xt = lnp.tile([P, D], F32, tag="xsrc")
                nc.sync.dma_start(xt, dram.ap()[ts(ti, P), :])
                return xt
            return get

        wpool = ctx.enter_context(tc.tile_pool(name="wpool", bufs=2))

        def load_hp_weights(hp):
            wq_t = wpool.tile([P, NDC, P], BF16, tag="wq", name=f"wq{hp}")
            nc.sync.dma_start(wq_t, views["wq_v"][:, :, ts(hp, P)])
            wk_t = wpool.tile([P, NDC, P], BF16, tag="wk", name=f"wk{hp}")
            nc.sync.dma_start(wk_t, views["wk_v"][:, :, ts(hp, P)])
            wko_t = wpool.tile([P, NDC, P], BF16, tag="wko", name=f"wko{hp}")
            nc.sync.dma_start(wko_t, views["wko_v"][:, :, ts(hp, P)])
            wva_t = wpool.tile([P, NDC, VW], BF16, tag="wva", name=f"wva{hp}")
            nc.sync.dma_start(wva_t, views["wva_v"][:, :, ts(hp, VW)])
            wvo_t = wpool.tile([P, NDC, VW], BF16, tag="wvo", name=f"wvo{hp}")
            nc.sync.dma_start(wvo_t, views["wvo_v"][:, :, ts(hp, VW)])
            return wq_t, wk_t, wko_t, wva_t, wvo_t

        with tc.tile_pool(name="tpsA", bufs=3, space="PSUM") as tpsA:
            layernorm_T(mk_src(dt_in["xin"]), g1_t, b1_t,
                        lambda ti: xnT_own_h[ti // 4], tpsA)
            preloaded = {0: load_hp_weights(0)}
            layernorm_T(mk_src(dt_in["oin"]), g1o_t, b1o_t,
                        lambda ti: xnT_oth_h[ti // 4], tpsA, cb_act=False)

        # ---------- Phase B+C: head-pair projections + attention ----------
        with ExitStack() as bc:
            qkv_pool = bc.enter_context(tc.tile_pool(name="qkv", bufs=2))
            probs_pool = bc.enter_context(tc.tile_pool(name="probs", bufs=3))
            small = bc.enter_context(tc.tile_pool(name="small", bufs=2))
            tnp = bc.enter_context(tc.tile_pool(name="tnp", bufs=1))
            proj_ps = bc.enter_context(
                tc.tile_pool(name="proj_ps", bufs=2, space="PSUM"))
            score_ps = bc.enter_context(
                tc.tile_pool(name="score_ps", bufs=2, space="PSUM"))
            ctx_ps = bc.enter_context(
                tc.tile_pool(name="ctx_ps", bufs=2, space="PSUM"))

            for hp in range(NHP):
                wq_t, wk_t, wko_t, wva_t, wvo_t = preloaded.pop(hp)
                if hp + 1 < NHP:
                    preloaded[hp + 1] = load_hp_weights(hp + 1)
                if hp == 1:
                    nc.sync.dma_start(w2s[:, 0:8, :], views["w2_v"][:, 0:8, :])
                elif hp == 2:
                    nc.sync.dma_start(w2s[:, 8:16, :], views["w2_v"][:, 8:16, :])
                elif hp == 3:
                    nc.sync.dma_start(wo_t, views["wo_v"][:])
                elif hp == 5:
                    nc.sync.dma_start(w2s[:, 16:24, :], views["w2_v"][:, 16:24, :])

                def proj_T(w_t, bias_col, xnT_h, tag):
                    res = qkv_pool.tile([P, S], BF16, tag=tag)
                    for qc in range(NQC):
                        ps = proj_ps.tile([P, QC], F32, tag="proj")
                        for c in range(NDC):
                            nc.tensor.matmul(ps, w_t[:, c, :],
                                             xnT_h[qc][:, c, :],
                                             start=(c == 0), stop=(c == NDC - 1))
                        nc.vector.tensor_scalar_add(
                            res[:, ds(qc * QC, QC)], ps, bias_col[:, hp:hp + 1])
                    return res

                qT = proj_T(wq_t, bq_t, xnT_own_h, "qT")
                kT = proj_T(wk_t, bk_t, xnT_own_h, "kT")
                kdT = proj_T(wko_t, bko_t, xnT_oth_h, "kdT")

                def proj_V(w_t, bias_bc, xnT_h, tag):
                    res = qkv_pool.tile([P, NT, VW], BF16, tag=tag)
                    for ti in range(NT):
                        psf = proj_ps.tile([P, QC], F32, tag="proj", name="projv_ps")
                        ps = psf[:, :VW]
                        for c in range(NDC):
                            nc.tensor.matmul(ps, xnT_h[ti // 4][:, c, ts(ti % 4, P)],
                                             w_t[:, c, :],
                                             start=(c == 0), stop=(c == NDC - 1))
                        nc.vector.tensor_tensor(res[:, ti, :], ps,
                                                bias_bc[:, ts(hp, VW)], ALU.add)
                    return res

                v_t = proj_V(wva_t, bva_t, xnT_own_h, "v")
                vd_t = proj_V(wvo_t, bvo_t, xnT_oth_h, "vd")

                if "noattn" in ABL:
                    continue
                for qc in range(NQC):
                    # normalized per-(attn, head) context staging tiles
                    tn = {}
                    for ai, (kk, vv) in enumerate(((kT, v_t), (kdT, vd_t))):
                        # scores for both heads issued adjacently: disjoint
                        # 64-row groups run concurrently on the PE sub-arrays
                        probs2 = probs_pool.tile([P, NT, 2, QC], BF16, tag="probs")
                        for ti in range(NT):
                            sps = score_ps.tile([P, 2, QC], F32, tag="sps")
                            for hl in range(2):
                                nc.tensor.matmul(sps[:, hl, :],
                                                 kk[hl * DH:(hl + 1) * DH, ts(ti, P)],
                                                 qT[hl * DH:(hl + 1) * DH,
                                                    ds(qc * QC, QC)],
                                                 start=True, stop=True)
                            nc.scalar.activation(out=probs2[:, ti, :, :], in_=sps,
                                                 func=AF.Exp, scale=0.125)
                        for hl in range(2):
                            cps = ctx_ps.tile([VAUG, QC], F32, tag="cps")
                            for ti in range(NT):
                                nc.tensor.matmul(cps, vv[:, ti, ds(hl * VAUG, VAUG)],
                                                 probs2[:, ti, hl, :],
                                                 start=(ti == 0), stop=(ti == NT - 1))
                            rec = small.tile([1, QC], F32, tag="rec")
                            nc.vector.reciprocal(out=rec, in_=cps[DH:VAUG, :])
                            bps = small.tile([DH, QC], F32, tag="bps")
                            nc.gpsimd.partition_broadcast(bps, rec)
                            t_n = tnp.tile([DH, QC], F32, tag=f"tn_{ai}_{hl}",
                                           name=f"tn_{ai}_{hl}")
                            nc.vector.tensor_tensor(t_n, cps[0:DH, :], bps, ALU.mult)
                            tn[(ai, hl)] = t_n
                    for hl in range(2):
                        nc.gpsimd.tensor_tensor(
                            ctxT[ds(hl * DH, DH), hp, ds(qc * QC, QC)],
                            tn[(0, hl)], tn[(1, hl)], ALU.add)

        # ---------- Phase D: out projection + residual -> x2 ----------
        acts2 = ctx.enter_context(tc.tile_pool(name="acts2", bufs=1))
        x2 = acts2.tile([P, NT, D], F32, tag="x2")
        with ExitStack() as dc:
            dpool = dc.enter_context(tc.tile_pool(name="dpool", bufs=4))
            ops_ps = dc.enter_context(
                tc.tile_pool(name="ops_ps", bufs=3, space="PSUM"))
            for ti in range(NT):
                rx = dpool.tile([P, D], F32, tag="rx")
                nc.sync.dma_start(rx, dt_in["xin"].ap()[ts(ti, P), :])
                for half in range(2):
                    ps = ops_ps.tile([P, 384], F32, tag="ops")
                    for c in range(NDC):
                        nc.tensor.matmul(ps, ctxT[:, c, ts(ti, P)],
                                         wo_t[:, c, ds(half * 384, 384)],
                                         start=(c == 0), stop=(c == NDC - 1))
                    tmp = dpool.tile([P, 384], F32, tag="otmp")
                    nc.vector.tensor_tensor(tmp, ps,
                                            boeff_t[:, ds(half * 384, 384)], ALU.add)
                    nc.gpsimd.tensor_tensor(x2[:, ti, ds(half * 384, 384)],
                                            tmp, rx[:, ds(half * 384, 384)], ALU.add)

        # ---------- Phase E: LN2 -> xn2T ----------
        xn2T_h = [acts2.tile([P, NDC, QC], BF16, tag=f"xn2T{h}",
                             name=f"xn2T{h}") for h in range(2)]
        with tc.tile_pool(name="tpsE", bufs=3, space="PSUM") as tpsE:
            layernorm_T(lambda ti: x2[:, ti, :], g2_t, b2_t,
                        lambda ti: xn2T_h[ti // 4], tpsE)

        if "nomlp" in ABL:
            with tc.tile_pool(name="skipf", bufs=2) as skipf:
                for ti in range(NT):
                    ot = skipf.tile([P, D], F32, tag="so")
                    nc.vector.tensor_copy(ot, x2[:, ti, :])
                    nc.sync.dma_start(out.ap()[ts(ti, P), :], ot)
            return
        # ---------- Phase F: MLP + residual -> out ----------
        with ExitStack() as fc:
            fwp = fc.enter_context(tc.tile_pool(name="fwp", bufs=1))
            hpool = fc.enter_context(tc.tile_pool(name="hpool", bufs=3))
            fepil = fc.enter_context(tc.tile_pool(name="fepil", bufs=3))
            mlp_ps = fc.enter_context(
                tc.tile_pool(name="mlp_ps", bufs=4, space="PSUM"))
            fc2_ps = fc.enter_context(
                tc.tile_pool(name="fc2_ps", bufs=1, space="PSUM"))

            w1pool = fc.enter_context(tc.tile_pool(name="w1pool", bufs=4))

            # Two S-half passes: fc1 at N=512 with h1 staged in SBUF, then
            # fc2 in two Dout halves of 4 PSUM banks each. Halves W1 traffic.
            h1store = fc.enter_context(tc.tile_pool(name="h1store", bufs=1))
            foutp = fc.enter_context(tc.tile_pool(name="foutp", bufs=1))
            for half in range(2):
                h1g = h1store.tile([P, NF, QC], BF16, tag="h1g")
                for fi in range(NF):
                    w1t = w1pool.tile([P, NDC, P], BF16, tag="w1t")
                    nc.sync.dma_start(w1t, views["w1_v"][:, :, ts(fi, P)])
                    f1ps = mlp_ps.tile([P, QC], F32, tag="f1")
                    for c in range(NDC):
                        nc.tensor.matmul(f1ps, w1t[:, c, :],
                                         xn2T_h[half][:, c, :],
                                         start=(c == 0), stop=(c == NDC - 1))
                    nc.scalar.activation(out=h1g[:, fi, :], in_=f1ps, func=AF.Gelu,
                                         bias=b1f_t[:, fi:fi + 1], scale=1.0)
                ots = [foutp.tile([P, D], F32, tag=f"fout{i}", name=f"fout{i}")
                       for i in range(4)]
                for dh in range(2):
                    f2ps = [fc2_ps.tile([P, 384], F32, tag=f"f2_{i}",
                                        name=f"f2ps_{i}") for i in range(4)]
                    for fi in range(NF):
                        for i in range(4):
                            nc.tensor.matmul(f2ps[i], h1g[:, fi, ts(i, P)],
                                             w2s[:, fi, ds(dh * 384, 384)],
                                             start=(fi == 0), stop=(fi == NF - 1))
                    for i in range(4):
                        ti = half * 4 + i
                        tmp = fepil.tile([P, 384], F32, tag="ftmp")
                        nc.vector.tensor_tensor(tmp, f2ps[i],
                                                b2f_t[:, ds(dh * 384, 384)], ALU.add)
                        nc.gpsimd.tensor_tensor(ots[i][:, ds(dh * 384, 384)], tmp,
                                                x2[:, ti, ds(dh * 384, 384)], ALU.add)
                        if dh == 1:
                            nc.sync.dma_start(out.ap()[ts(ti, P), :], ots[i])


_NC_CACHE = None


def _get_program():
    global _NC_CACHE
    if _NC_CACHE is None:
        _NC_CACHE = build_program()
    return _NC_CACHE


def _col(v):
    """[k*128] -> [128, k] per-partition col layout (f = c*128 + p)."""
    return np.ascontiguousarray(np.asarray(v, np.float32).reshape(-1, P).T)


def _bf(a):
    return np.ascontiguousarray(np.asarray(a).astype(ml_dtypes.bfloat16))


def make_core_inputs(x, y, params):
    p = params
    scal = {k: float(np.asarray(p[k]).reshape(())) for k in
            ("w11", "w12", "w21", "w22")}

    def vaug_pack(wv, bv, scale):
        wvs = np.asarray(wv, np.float32) * scale
        bvs = np.asarray(bv, np.float32) * scale
        wout = np.zeros((D, H * VAUG), np.float32)
        bout = np.zeros((H * VAUG,), np.float32)
        for h in range(H):
            wout[:, h * VAUG:h * VAUG + DH] = wvs[:, h * DH:(h + 1) * DH]
            bout[h * VAUG:h * VAUG + DH] = bvs[h * DH:(h + 1) * DH]
            bout[h * VAUG + DH] = 1.0
        return wout, bout

    ins = []
    for c in range(N_CORES):
        stream, b = c // 4, c % 4
        if stream == 0:
            xin, oin = x[b], y[b]
            g1, b1, g1o, b1o = p["ln_a_g"], p["ln_a_b"], p["ln_ad_g"], p["ln_ad_b"]
            g2, b2 = p["ln_f_g"], p["ln_f_b"]
            Wq, bq, Wk, bk = p["Wq"], p["bq"], p["Wk"], p["bk"]
            Wv, bv = p["Wv"], p["bv"]
            Wko, bko, Wvo, bvo = p["Wkd"], p["bkd"], p["Wvd"], p["bvd"]
            Wo, bo = p["Wo"], p["bo"]
            ws, wc = scal["w11"], scal["w12"]
            W1, B1, W2, B2 = p["fc1_w"], p["fc1_b"], p["fc2_w"], p["fc2_b"]
        else:
            xin, oin = y[b], x[b]
            g1, b1, g1o, b1o = p["ln_ad_g"], p["ln_ad_b"], p["ln_a_g"], p["ln_a_b"]
            g2, b2 = p["ln_fd_g"], p["ln_fd_b"]
            Wq, bq, Wk, bk = p["Wqd"], p["bqd"], p["Wkd"], p["bkd"]
            Wv, bv = p["Wvd"], p["bvd"]
            Wko, bko, Wvo, bvo = p["Wk"], p["bk"], p["Wv"], p["bv"]
            Wo, bo = p["Wod"], p["bod"]
            ws, wc = scal["w21"], scal["w22"]
            W1, B1, W2, B2 = p["fc1d_w"], p["fc1d_b"], p["fc2d_w"], p["fc2d_b"]

        wva, bva = vaug_pack(Wv, bv, ws)
        wvo, bvo_ = vaug_pack(Wvo, bvo, wc)
        cols = np.concatenate(
            [_col(v) for v in (g1, b1, g1o, b1o, g2, b2, bq, bk, bko, B1)], axis=1)
        brows = np.concatenate(
            [bva, bvo_, (ws + wc) * np.asarray(bo, np.float32),
             np.asarray(B2, np.float32)])
        ins.append({
            "xin": np.ascontiguousarray(np.asarray(xin, np.float32)),
            "oin": np.ascontiguousarray(np.asarray(oin, np.float32)),
            "cols": np.ascontiguousarray(cols.astype(np.float32)),
            "brows": np.ascontiguousarray(brows.astype(np.float32)),
            "wq": _bf(Wq), "wk": _bf(Wk), "wko": _bf(Wko),
            "wvaug": _bf(wva), "wvoaug": _bf(wvo),
            "wo": _bf(Wo),
            "w1": _bf(W1), "w2": _bf(W2),
        })
    return ins


def kernel(x, y, params):
    from concourse.bass_utils import run_bass_kernel_spmd

    nc = _get_program()
    ins = make_core_inputs(np.asarray(x), np.asarray(y), params)
    res = run_bass_kernel_spmd(nc, ins, list(range(N_CORES)))
    full = np.empty((2, B, S, D), np.float32)
    for c in range(N_CORES):
        full[c // 4, c % 4] = res.results[c]["out"]
    return full


# revision 28
# speedup vs baseline: 1.0009x; 1.0009x over previous
"""Trainium2 Bass kernel for the dual-stream transformer block.

Shapes (hardcoded): B=4, S=1024, D=768, F=3072, H=12, DH=64, 8 cores.
Sharding: data-parallel; core c handles (stream = c//4, batch = c%4) and
computes that stream's full [S, D] output. Cross-attention K/V for the other
stream is recomputed locally (no collectives). The w11/w12-style stream-mix
scalars are folded into the V projection weights host-side (linearity), and
softmax normalizers come from a ones-column augmented V.
"""
import sys, os

sys.path.insert(0, "/opt/trn_rl_repo")

ABL = set(os.environ.get("ABL", "").split(",")) - {""}
NREP = int(os.environ.get("NREP", "1"))

import numpy as np
import ml_dtypes

import concourse.bass as bass
import concourse.mybir as mybir
import concourse.tile as tile
from concourse import bacc
from concourse.bass import ts, ds
from concourse.masks import make_identity

F32 = mybir.dt.float32
BF16 = mybir.dt.bfloat16
AF = mybir.ActivationFunctionType
ALU = mybir.AluOpType

B = 4
S = 1024
D = 768
FF = 3072
H = 12
DH = 64
P = 128
NDC = D // P          # 6 chunks of D
NF = FF // P          # 24 chunks of F
NT = S // P           # 8 token tiles
NHP = H // 2          # 6 head pairs
QC = 512              # q-chunk (attention moving free dim)
NQC = S // QC         # 2
EPS = 1e-6
N_CORES = 8

VAUG = DH + 1         # 65: v columns + ones column
VW = 2 * VAUG         # 130 per head pair


def _bcast_ap(ap, n):
    """Partition-broadcast read AP: replicate across n partitions."""
    dims = list(ap.ap)
    if dims and dims[0][1] == 1:
        dims = dims[1:]
    return bass.AP(tensor=ap.tensor, offset=ap.offset, ap=[[0, n]] + dims)


def build_program():
    nc = bacc.Bacc("TRN2", target_bir_lowering=False, debug=False,
                   num_devices=N_CORES)

    dt_in = {}

    def din(name, shape, dt=F32):
        h = nc.dram_tensor(name, shape, dt, kind="ExternalInput")
        dt_in[name] = h
        return h

    din("xin", [S, D])
    din("oin", [S, D])
    din("cols", [P, 9 * NDC + NF])
    din("wq", [D, D], BF16)
    din("wk", [D, D], BF16)
    din("wko", [D, D], BF16)
    din("wvaug", [D, H * VAUG], BF16)
    din("wvoaug", [D, H * VAUG], BF16)

    din("wo", [D, D], BF16)

    din("w1", [D, FF], BF16)

    din("w2", [FF, D], BF16)
    din("brows", [2 * H * VAUG + 2 * D])
    out = nc.dram_tensor("out", [S, D], F32, kind="ExternalOutput")

    views = {
        "wq_v": dt_in["wq"].ap().rearrange("(c kp) n -> kp c n", kp=P),
        "wk_v": dt_in["wk"].ap().rearrange("(c kp) n -> kp c n", kp=P),
        "wko_v": dt_in["wko"].ap().rearrange("(c kp) n -> kp c n", kp=P),
        "wva_v": dt_in["wvaug"].ap().rearrange("(c kp) n -> kp c n", kp=P),
        "wvo_v": dt_in["wvoaug"].ap().rearrange("(c kp) n -> kp c n", kp=P),
        "wo_v": dt_in["wo"].ap().rearrange("(c kp) n -> kp c n", kp=P),
        "w1_v": dt_in["w1"].ap().rearrange("(c kp) n -> kp c n", kp=P),
        "w2_v": dt_in["w2"].ap().rearrange("(c kp) n -> kp c n", kp=P),
    }

    with tile.TileContext(nc) as tc:
        for _rep in range(NREP):
            _emit(nc, tc, dt_in, views, out)
    nc.compile()
    return nc


def _emit(nc, tc, dt_in, views, out):
    from contextlib import ExitStack

    with ExitStack() as ctx:
        glob = ctx.enter_context(tc.tile_pool(name="glob", bufs=1))
        acts1 = None  # opened below, closed after phase D
        lnp = ctx.enter_context(tc.tile_pool(name="lnp", bufs=2))
        lns = ctx.enter_context(tc.tile_pool(name="lns", bufs=4))

        ident = glob.tile([P, P], BF16)
        make_identity(nc, ident)
        eps_t = glob.tile([P, 1], F32)
        nc.vector.memset(eps_t, EPS)
        ones_t = glob.tile([1, DH], BF16)
        nc.vector.memset(ones_t, 1.0)

        cols_t = glob.tile([P, 9 * NDC + NF], F32)
        nc.sync.dma_start(cols_t, dt_in["cols"].ap())
        g1_t, b1_t = cols_t[:, 0:6], cols_t[:, 6:12]
        g1o_t, b1o_t = cols_t[:, 12:18], cols_t[:, 18:24]
        g2_t, b2_t = cols_t[:, 24:30], cols_t[:, 30:36]
        bq_t, bk_t, bko_t = cols_t[:, 36:42], cols_t[:, 42:48], cols_t[:, 48:54]
        b1f_t = cols_t[:, 54:78]

        brows_t = glob.tile([P, 2 * H * VAUG + 2 * D], F32)
        nc.sync.dma_start(brows_t, _bcast_ap(dt_in["brows"].ap(), P))
        HV = H * VAUG
        bva_t = brows_t[:, 0:HV]
        bvo_t = brows_t[:, HV:2 * HV]
        boeff_t = brows_t[:, 2 * HV:2 * HV + D]
        b2f_t = brows_t[:, 2 * HV + D:2 * HV + 2 * D]

        w2p = ctx.enter_context(tc.tile_pool(name="w2p", bufs=1))
        w2s = w2p.tile([P, NF, D], BF16)
        dwp = ctx.enter_context(tc.tile_pool(name="dwp", bufs=1))
        wo_t = dwp.tile([P, NDC, D], BF16)

        # ---------- LayerNorm helper (token-major stats, transposed output) ----
        def ln_tile(ti, src_getter, g_t, b_t, xnT, tps, cb_act):
            if True:
                xt = src_getter(ti)
                stats = lns.tile([P, 3, 6], F32, tag="stats")
                for sg in range(3):
                    nc.vector.bn_stats(out=stats[:, sg, :],
                                       in_=xt[:, ds(sg * 256, 256)])
                mv = lns.tile([P, 2], F32, tag="mv")
                nc.vector.bn_aggr(out=mv, in_=stats)
                rstd = lns.tile([P, 1], F32, tag="rstd")
                nc.scalar.activation(out=rstd, in_=mv[:, 1:2], func=AF.Sqrt,
                                     bias=eps_t, scale=1.0)
                nc.vector.reciprocal(out=rstd, in_=rstd)
                nmr = lns.tile([P, 1], F32, tag="nmr")
                nc.vector.tensor_tensor(nmr, mv[:, 0:1], rstd, ALU.mult)
                nc.vector.tensor_scalar_mul(nmr, nmr, -1.0)
                xn = lnp.tile([P, D], BF16, tag="xn")
                nc.scalar.activation(out=xn, in_=xt, func=AF.Identity,
                                     bias=nmr, scale=rstd)
                for c in range(NDC):
                    ps = tps.tile([P, P], BF16, tag="tp")
                    nc.tensor.transpose(ps, xn[:, ts(c, P)], ident)
                    if c % 2 == (0 if cb_act else 1):
                        nc.scalar.activation(
                            out=xnT(ti)[:, c, ts(ti % 4, P)], in_=ps,
                            func=AF.Identity, bias=b_t[:, c:c + 1],
                            scale=g_t[:, c:c + 1])
                    else:
                        nc.vector.tensor_scalar(
                            out=xnT(ti)[:, c, ts(ti % 4, P)], in0=ps,
                            scalar1=g_t[:, c:c + 1], scalar2=b_t[:, c:c + 1],
                            op0=ALU.mult, op1=ALU.add)

        def layernorm_T(src_getter, g_t, b_t, xnT, tps, cb_act=True):
            for ti in range(NT):
                ln_tile(ti, src_getter, g_t, b_t, xnT, tps, cb_act)

        # ---------- Phase A: LN1 for both streams ----------
        # xnT stored as two S-half tiles so consumers can start after half LN
        acts1 = ctx.enter_context(tc.tile_pool(name="acts1", bufs=1))
        xnT_own_h = [acts1.tile([P, NDC, QC], BF16, tag=f"xnT_own{h}",
                                name=f"xnT_own{h}") for h in range(2)]
        xnT_oth_h = [acts1.tile([P, NDC, QC], BF16, tag=f"xnT_oth{h}",
                                name=f"xnT_oth{h}") for h in range(2)]
        ctxT = acts1.tile([P, NDC, S], BF16, tag="ctxT")

        def mk_src(dram):
            def get(ti):
                xt = lnp.tile([P, D], F32, tag="xsrc")
                nc.sync.dma_start(xt, dram.ap()[ts(ti, P), :])
                return xt
            return get

        wpool = ctx.enter_context(tc.tile_pool(name="wpool", bufs=2))

        def load_hp_weights(hp):
            wq_t = wpool.tile([P, NDC, P], BF16, tag="wq", name=f"wq{hp}")
            nc.sync.dma_start(wq_t, views["wq_v"][:, :, ts(hp, P)])
            wk_t = wpool.tile([P, NDC, P], BF16, tag="wk", name=f"wk{hp}")
            nc.sync.dma_start(wk_t, views["wk_v"][:, :, ts(hp, P)])
            wko_t = wpool.tile([P, NDC, P], BF16, tag="wko", name=f"wko{hp}")
            nc.sync.dma_start(wko_t, views["wko_v"][:, :, ts(hp, P)])
            wva_t = wpool.tile([P, NDC, VW], BF16, tag="wva", name=f"wva{hp}")
            nc.sync.dma_start(wva_t, views["wva_v"][:, :, ts(hp, VW)])
            wvo_t = wpool.tile([P, NDC, VW], BF16, tag="wvo", name=f"wvo{hp}")
            nc.sync.dma_start(wvo_t, views["wvo_v"][:, :, ts(hp, VW)])
            return wq_t, wk_t, wko_t, wva_t, wvo_t

        with tc.tile_pool(name="tpsA", bufs=3, space="PSUM") as tpsA:
            src_own, src_oth = mk_src(dt_in["xin"]), mk_src(dt_in["oin"])
            for ti in range(NT):
                ln_tile(ti, src_own, g1_t, b1_t,
                        lambda t: xnT_own_h[t // 4], tpsA, True)
                ln_tile(ti, src_oth, g1o_t, b1o_t,
                        lambda t: xnT_oth_h[t // 4], tpsA, False)
                if ti == 3:
                    preloaded = {0: load_hp_weights(0)}

        # ---------- Phase B+C: head-pair projections + attention ----------
        with ExitStack() as bc:
            qkv_pool = bc.enter_context(tc.tile_pool(name="qkv", bufs=2))
            probs_pool = bc.enter_context(tc.tile_pool(name="probs", bufs=3))
            small = bc.enter_context(tc.tile_pool(name="small", bufs=2))
            tnp = bc.enter_context(tc.tile_pool(name="tnp", bufs=1))
            proj_ps = bc.enter_context(
                tc.tile_pool(name="proj_ps", bufs=2, space="PSUM"))
            score_ps = bc.enter_context(
                tc.tile_pool(name="score_ps", bufs=2, space="PSUM"))
            ctx_ps = bc.enter_context(
                tc.tile_pool(name="ctx_ps", bufs=2, space="PSUM"))

            for hp in range(NHP):
                wq_t, wk_t, wko_t, wva_t, wvo_t = preloaded.pop(hp)
                if hp + 1 < NHP:
                    preloaded[hp + 1] = load_hp_weights(hp + 1)
                if hp == 1:
                    nc.sync.dma_start(w2s[:, 0:8, :], views["w2_v"][:, 0:8, :])
                elif hp == 2:
                    nc.sync.dma_start(w2s[:, 8:16, :], views["w2_v"][:, 8:16, :])
                elif hp == 3:
                    nc.sync.dma_start(wo_t, views["wo_v"][:])
                elif hp == 5:
                    nc.sync.dma_start(w2s[:, 16:24, :], views["w2_v"][:, 16:24, :])

                def proj_T(w_t, bias_col, xnT_h, tag):
                    res = qkv_pool.tile([P, S], BF16, tag=tag)
                    for qc in range(NQC):
                        ps = proj_ps.tile([P, QC], F32, tag="proj")
                        for c in range(NDC):
                            nc.tensor.matmul(ps, w_t[:, c, :],
                                             xnT_h[qc][:, c, :],
                                             start=(c == 0), stop=(c == NDC - 1))
                        nc.vector.tensor_scalar_add(
                            res[:, ds(qc * QC, QC)], ps, bias_col[:, hp:hp + 1])
                    return res

                qT = proj_T(wq_t, bq_t, xnT_own_h, "qT")
                kT = proj_T(wk_t, bk_t, xnT_own_h, "kT")
                kdT = proj_T(wko_t, bko_t, xnT_oth_h, "kdT")

                def proj_V(w_t, bias_bc, xnT_h, tag):
                    res = qkv_pool.tile([P, NT, VW], BF16, tag=tag)
                    for ti in range(NT):
                        psf = proj_ps.tile([P, QC], F32, tag="proj", name="projv_ps")
                        ps = psf[:, :VW]
                        for c in range(NDC):
                            nc.tensor.matmul(ps, xnT_h[ti // 4][:, c, ts(ti % 4, P)],
                                             w_t[:, c, :],
                                             start=(c == 0), stop=(c == NDC - 1))
                        nc.vector.tensor_tensor(res[:, ti, :], ps,
                                                bias_bc[:, ts(hp, VW)], ALU.add)
                    return res

                v_t = proj_V(wva_t, bva_t, xnT_own_h, "v")
                vd_t = proj_V(wvo_t, bvo_t, xnT_oth_h, "vd")

                if "noattn" in ABL:
                    continue
                for qc in range(NQC):
                    # normalized per-(attn, head) context staging tiles
                    tn = {}
                    for ai, (kk, vv) in enumerate(((kT, v_t), (kdT, vd_t))):
                        # scores for both heads issued adjacently: disjoint
                        # 64-row groups run concurrently on the PE sub-arrays
                        probs2 = probs_pool.tile([P, NT, 2, QC], BF16, tag="probs")
                        for ti in range(NT):
                            sps = score_ps.tile([P, 2, QC], F32, tag="sps")
                            for hl in range(2):
                                nc.tensor.matmul(sps[:, hl, :],
                                                 kk[hl * DH:(hl + 1) * DH, ts(ti, P)],
                                                 qT[hl * DH:(hl + 1) * DH,
                                                    ds(qc * QC, QC)],
                                                 start=True, stop=True)
                            nc.scalar.activation(out=probs2[:, ti, :, :], in_=sps,
                                                 func=AF.Exp, scale=0.125)
                        for hl in range(2):
                            cps = ctx_ps.tile([VAUG, QC], F32, tag="cps")
                            for ti in range(NT):
                                nc.tensor.matmul(cps, vv[:, ti, ds(hl * VAUG, VAUG)],
                                                 probs2[:, ti, hl, :],
                                                 start=(ti == 0), stop=(ti == NT - 1))
                            rec = small.tile([1, QC], F32, tag="rec")
                            nc.vector.reciprocal(out=rec, in_=cps[DH:VAUG, :])
                            bps = small.tile([DH, QC], F32, tag="bps")
                            nc.gpsimd.partition_broadcast(bps, rec)
                            t_n = tnp.tile([DH, QC], F32, tag=f"tn_{ai}_{hl}",
                                           name=f"tn_{ai}_{hl}")
                            nc.vector.tensor_tensor(t_n, cps[0:DH, :], bps, ALU.mult)
                            tn[(ai, hl)] = t_n
                    for hl in range(2):
                        nc.gpsimd.tensor_tensor(
                            ctxT[ds(hl * DH, DH), hp, ds(qc * QC, QC)],
                            tn[(0, hl)], tn[(1, hl)], ALU.add)

        # ---------- Phase D: out projection + residual -> x2 ----------
        acts2 = ctx.enter_context(tc.tile_pool(name="acts2", bufs=1))
        x2 = acts2.tile([P, NT, D], F32, tag="x2")
        with ExitStack() as dc:
            dpool = dc.enter_context(tc.tile_pool(name="dpool", bufs=4))
            ops_ps = dc.enter_context(
                tc.tile_pool(name="ops_ps", bufs=3, space="PSUM"))
            for ti in range(NT):
                rx = dpool.tile([P, D], F32, tag="rx")
                nc.sync.dma_start(rx, dt_in["xin"].ap()[ts(ti, P), :])
                for half in range(2):
                    ps = ops_ps.tile([P, 384], F32, tag="ops")
                    for c in range(NDC):
                        nc.tensor.matmul(ps, ctxT[:, c, ts(ti, P)],
                                         wo_t[:, c, ds(half * 384, 384)],
                                         start=(c == 0), stop=(c == NDC - 1))
                    tmp = dpool.tile([P, 384], F32, tag="otmp")
                    nc.vector.tensor_tensor(tmp, ps,
                                            boeff_t[:, ds(half * 384, 384)], ALU.add)
                    nc.gpsimd.tensor_tensor(x2[:, ti, ds(half * 384, 384)],
                                            tmp, rx[:, ds(half * 384, 384)], ALU.add)

        # ---------- Phase E: LN2 -> xn2T ----------
        xn2T_h = [acts2.tile([P, NDC, QC], BF16, tag=f"xn2T{h}",
                             name=f"xn2T{h}") for h in range(2)]
        with tc.tile_pool(name="tpsE", bufs=3, space="PSUM") as tpsE:
            layernorm_T(lambda ti: x2[:, ti, :], g2_t, b2_t,
                        lambda ti: xn2T_h[ti // 4], tpsE)

        if "nomlp" in ABL:
            with tc.tile_pool(name="skipf", bufs=2) as skipf:
                for ti in range(NT):
                    ot = skipf.tile([P, D], F32, tag="so")
                    nc.vector.tensor_copy(ot, x2[:, ti, :])
                    nc.sync.dma_start(out.ap()[ts(ti, P), :], ot)
            return
        # ---------- Phase F: MLP + residual -> out ----------
        with ExitStack() as fc:
            fwp = fc.enter_context(tc.tile_pool(name="fwp", bufs=1))
            hpool = fc.enter_context(tc.tile_pool(name="hpool", bufs=3))
            fepil = fc.enter_context(tc.tile_pool(name="fepil", bufs=3))
            mlp_ps = fc.enter_context(
                tc.tile_pool(name="mlp_ps", bufs=4, space="PSUM"))
            fc2_ps = fc.enter_context(
                tc.tile_pool(name="fc2_ps", bufs=1, space="PSUM"))

            w1pool = fc.enter_context(tc.tile_pool(name="w1pool", bufs=4))

            # Two S-half passes: fc1 at N=512 with h1 staged in SBUF, then
            # fc2 in two Dout halves of 4 PSUM banks each. Halves W1 traffic.
            h1store = fc.enter_context(tc.tile_pool(name="h1store", bufs=1))
            foutp = fc.enter_context(tc.tile_pool(name="foutp", bufs=1))
            for half in range(2):
                h1g = h1store.tile([P, NF, QC], BF16, tag="h1g")
                for fi in range(NF):
                    w1t = w1pool.tile([P, NDC, P], BF16, tag="w1t")
                    nc.sync.dma_start(w1t, views["w1_v"][:, :, ts(fi, P)])
                    f1ps = mlp_ps.tile([P, QC], F32, tag="f1")
                    for c in range(NDC):
                        nc.tensor.matmul(f1ps, w1t[:, c, :],
                                         xn2T_h[half][:, c, :],
                                         start=(c == 0), stop=(c == NDC - 1))
                    nc.scalar.activation(out=h1g[:, fi, :], in_=f1ps, func=AF.Gelu,
                                         bias=b1f_t[:, fi:fi + 1], scale=1.0)
                ots = [foutp.tile([P, D], F32, tag=f"fout{i}", name=f"fout{i}")
                       for i in range(4)]
                for dh in range(2):
                    f2ps = [fc2_ps.tile([P, 384], F32, tag=f"f2_{i}",
                                        name=f"f2ps_{i}") for i in range(4)]
                    for fi in range(NF):
                        for i in range(4):
                            nc.tensor.matmul(f2ps[i], h1g[:, fi, ts(i, P)],
                                             w2s[:, fi, ds(dh * 384, 384)],
                                             start=(fi == 0), stop=(fi == NF - 1))
                    for i in range(4):
                        ti = half * 4 + i
                        tmp = fepil.tile([P, 384], F32, tag="ftmp")
                        nc.vector.tensor_tensor(tmp, f2ps[i],
                                                b2f_t[:, ds(dh * 384, 384)], ALU.add)
                        nc.gpsimd.tensor_tensor(ots[i][:, ds(dh * 384, 384)], tmp,
                                                x2[:, ti, ds(dh * 384, 384)], ALU.add)
                        if dh == 1:
                            nc.sync.dma_start(out.ap()[ts(ti, P), :], ots[i])


_NC_CACHE = None


def _get_program():
    global _NC_CACHE
    if _NC_CACHE is None:
        _NC_CACHE = build_program()
    return _NC_CACHE


def _col(v):
    """[k*128] -> [128, k] per-partition col layout (f = c*128 + p)."""
    return np.ascontiguousarray(np.asarray(v, np.float32).reshape(-1, P).T)


def _bf(a):
    return np.ascontiguousarray(np.asarray(a).astype(ml_dtypes.bfloat16))


def make_core_inputs(x, y, params):
    p = params
    scal = {k: float(np.asarray(p[k]).reshape(())) for k in
            ("w11", "w12", "w21", "w22")}

    def vaug_pack(wv, bv, scale):
        wvs = np.asarray(wv, np.float32) * scale
        bvs = np.asarray(bv, np.float32) * scale
        wout = np.zeros((D, H * VAUG), np.float32)
        bout = np.zeros((H * VAUG,), np.float32)
        for h in range(H):
            wout[:, h * VAUG:h * VAUG + DH] = wvs[:, h * DH:(h + 1) * DH]
            bout[h * VAUG:h * VAUG + DH] = bvs[h * DH:(h + 1) * DH]
            bout[h * VAUG + DH] = 1.0
        return wout, bout

    ins = []
    for c in range(N_CORES):
        stream, b = c // 4, c % 4
        if stream == 0:
            xin, oin = x[b], y[b]
            g1, b1, g1o, b1o = p["ln_a_g"], p["ln_a_b"], p["ln_ad_g"], p["ln_ad_b"]
            g2, b2 = p["ln_f_g"], p["ln_f_b"]
            Wq, bq, Wk, bk = p["Wq"], p["bq"], p["Wk"], p["bk"]
            Wv, bv = p["Wv"], p["bv"]
            Wko, bko, Wvo, bvo = p["Wkd"], p["bkd"], p["Wvd"], p["bvd"]
            Wo, bo = p["Wo"], p["bo"]
            ws, wc = scal["w11"], scal["w12"]
            W1, B1, W2, B2 = p["fc1_w"], p["fc1_b"], p["fc2_w"], p["fc2_b"]
        else:
            xin, oin = y[b], x[b]
            g1, b1, g1o, b1o = p["ln_ad_g"], p["ln_ad_b"], p["ln_a_g"], p["ln_a_b"]
            g2, b2 = p["ln_fd_g"], p["ln_fd_b"]
            Wq, bq, Wk, bk = p["Wqd"], p["bqd"], p["Wkd"], p["bkd"]
            Wv, bv = p["Wvd"], p["bvd"]
            Wko, bko, Wvo, bvo = p["Wk"], p["bk"], p["Wv"], p["bv"]
            Wo, bo = p["Wod"], p["bod"]
            ws, wc = scal["w21"], scal["w22"]
            W1, B1, W2, B2 = p["fc1d_w"], p["fc1d_b"], p["fc2d_w"], p["fc2d_b"]

        wva, bva = vaug_pack(Wv, bv, ws)
        wvo, bvo_ = vaug_pack(Wvo, bvo, wc)
        cols = np.concatenate(
            [_col(v) for v in (g1, b1, g1o, b1o, g2, b2, bq, bk, bko, B1)], axis=1)
        brows = np.concatenate(
            [bva, bvo_, (ws + wc) * np.asarray(bo, np.float32),
             np.asarray(B2, np.float32)])
        ins.append({
            "xin": np.ascontiguousarray(np.asarray(xin, np.float32)),
            "oin": np.ascontiguousarray(np.asarray(oin, np.float32)),
            "cols": np.ascontiguousarray(cols.astype(np.float32)),
            "brows": np.ascontiguousarray(brows.astype(np.float32)),
            "wq": _bf(Wq), "wk": _bf(Wk), "wko": _bf(Wko),
            "wvaug": _bf(wva), "wvoaug": _bf(wvo),
            "wo": _bf(Wo),
            "w1": _bf(W1), "w2": _bf(W2),
        })
    return ins


def kernel(x, y, params):
    from concourse.bass_utils import run_bass_kernel_spmd

    nc = _get_program()
    ins = make_core_inputs(np.asarray(x), np.asarray(y), params)
    res = run_bass_kernel_spmd(nc, ins, list(range(N_CORES)))
    full = np.empty((2, B, S, D), np.float32)
    for c in range(N_CORES):
        full[c // 4, c % 4] = res.results[c]["out"]
    return full
